# revision 14
# baseline (speedup 1.0000x reference)
"""GCN (6-layer GCNConv) Trainium2 Bass kernel — v2.

Data-parallel over batch (1 mesh per NeuronCore). Per layer
out = A_hat @ (x @ W) + b with A_hat = D^-1/2 (A+I) D^-1/2 shared across batch
and layers.

v2 structure (per core):
  - Host: symmetric-norm edge list WITHOUT self-loops (their contribution is
    added on-device as a PE transpose of the diag-scaled h tile, accumulated
    into the same PSUM segment-sum group). Nodes are relabeled (degree-balanced
    bin packing) so every 128-node dst tile has <= C*128 in-edges; edges are
    grouped per dst tile and padded to C chunks of 128.
  - Device: phases interleave scatter(i) with dense(i+1) per dst tile. The
    scatter's feature-major output tile (stage, SBUF) is consumed directly as
    the next dense matmul's lhsT — activations never round-trip through DRAM
    between layers; only the node-major h gather tables do.
  - Layer 1 uses the rank-1 structure of the broadcast image features:
    h1 = verts @ W1[:3] + (img @ W1[3:]) broadcast over nodes (host-computed).
  - Layer 5 scatter runs in orientation A (node-major out, self-loops kept as
    real edges) to produce the gather table for layer 6's message-first pass.
  - Layer 6: message passing first (64-wide), then the 64->3 matmul.
"""
import sys
import time

sys.path.insert(0, "/opt/trn_rl_repo")
import numpy as np
from contextlib import ExitStack

import concourse.bass as bass
import concourse.mybir as mybir
import concourse.tile as tile
from concourse.bass_utils import run_bass_kernel_spmd
from concourse.masks import make_identity

P = 128
F32 = mybir.dt.float32
I32 = mybir.dt.int32

_msw_ctr = [0]


def _split_multiwaits(nc, max_waits=1):
    """This walrus build rejects >1 sync wait per instruction: split extras
    onto preceding same-engine NOPs."""
    for f in nc.m.functions:
        for b in f.blocks:
            out, changed = [], False
            for inst in b.instructions:
                si = getattr(inst, "sync_info", None)
                waits = list(si.on_wait) if si is not None else []
                if len(waits) > max_waits:
                    changed = True
                    for w in waits[:-max_waits]:
                        _msw_ctr[0] += 1
                        nop = mybir.InstNoOp(name=f"msw-{_msw_ctr[0]}", ins=[], outs=[])
                        nop.engine = inst.engine
                        nop.sync_info = mybir.SyncInfo(on_wait=[w], on_update=[])
                        out.append(nop)
                    si.on_wait = waits[-max_waits:]
                out.append(inst)
            if changed:
                b.instructions = out
    return nc


def _pack_graph(src, dst, N):
    """Relabel nodes into degree-balanced 128-node tiles (no self-loops in the
    edge list). Returns device arrays [128, T*C] plus the with-self-loops
    variant [128, T*(C+1)] used by layer 5."""
    T = (N + P - 1) // P
    NP = T * P
    indeg = np.bincount(dst, minlength=N)          # no-loop in-degree
    C = max(1, int(np.ceil(len(src) / (T * P))))

    order = np.argsort(-indeg, kind="stable")
    while True:
        cap = C * P
        load = np.zeros(T, np.int64)
        count = np.zeros(T, np.int64)
        assign = np.empty(N, np.int64)
        ok = True
        for v in order:
            d = int(indeg[v])
            best_t, best_rem = -1, -1
            for t in range(T):
                if count[t] < P:
                    rem = cap - load[t]
                    if rem > best_rem:
                        best_rem, best_t = rem, t
            if best_t < 0 or load[best_t] + d > cap:
                ok = False
                break
            assign[v] = best_t
            load[best_t] += d
            count[best_t] += 1
        if ok:
            break
        C += 1

    perm = np.full(NP, -1, np.int64)
    new_of_old = np.empty(N, np.int64)
    cursor = np.zeros(T, np.int64)
    for v in range(N):
        t = assign[v]
        nid = t * P + cursor[t]
        cursor[t] += 1
        perm[nid] = v
        new_of_old[v] = nid

    # symmetric normalization (degree INCLUDES self-loops, per GCN)
    deg = (indeg + 1).astype(np.float32)
    dinv = (1.0 / np.sqrt(deg, dtype=np.float32)).astype(np.float32)
    norm = (dinv[src] * dinv[dst]).astype(np.float32)

    src_n = new_of_old[src]
    dst_n = new_of_old[dst]
    tile_of_e = dst_n // P
    order_e = np.argsort(tile_of_e, kind="stable")
    src_n, dst_n, norm = src_n[order_e], dst_n[order_e], norm[order_e]
    tile_of_e = tile_of_e[order_e]

    gsrc = np.zeros((T, C, P), np.int32)
    slot = np.zeros((T, C, P), np.float32)
    nrm = np.zeros((T, C, P), np.float32)
    starts = np.searchsorted(tile_of_e, np.arange(T + 1))
    for t in range(T):
        lo, hi = starts[t], starts[t + 1]
        n_e = hi - lo
        assert n_e <= C * P, (t, n_e, C * P)
        fs = np.zeros(C * P, np.int32)
        fl = np.zeros(C * P, np.float32)
        fn = np.zeros(C * P, np.float32)
        fs[:n_e] = src_n[lo:hi]
        fl[:n_e] = (dst_n[lo:hi] - t * P).astype(np.float32)
        fn[:n_e] = norm[lo:hi]
        gsrc[t] = fs.reshape(C, P)
        slot[t] = fl.reshape(C, P)
        nrm[t] = fn.reshape(C, P)

    # per-(slot, tile) dinv^2 for the on-device self-loop term (0 for dummies)
    dinv_new = np.zeros(NP, np.float32)
    valid = perm >= 0
    dinv_new[valid] = dinv[perm[valid]]
    dinv2 = (dinv_new ** 2).reshape(T, P).T.copy()   # [128, T]

    def dev(a):
        return np.ascontiguousarray(a.transpose(2, 0, 1).reshape(P, -1))

    return dict(NP=NP, T=T, C=C, perm=perm, dinv2=np.ascontiguousarray(dinv2),
                gsrc=dev(gsrc), slot=dev(slot), norm=dev(nrm))


def _build_nc(NP, T, C, FM, F5, FO):
    import os
    scratch = int(os.environ.get("KBASS_SCRATCH", "16384"))
    MD = mybir.dt.bfloat16 if os.environ.get("KBASS_MSGDT", "f32") == "bf16" else F32
    nc = bass.Bass(dynamic_dma_scratch_size=scratch)
    TC = T * C
    C5 = C + 1
    KM = FM // P
    K5 = FM // P

    d = {}
    d["xT1"] = nc.dram_tensor("xT1", [3, NP], F32, kind="ExternalInput")
    d["hcrep"] = nc.dram_tensor("hcrep", [P, FM], F32, kind="ExternalInput")
    d["W1v"] = nc.dram_tensor("W1v", [3, FM], F32, kind="ExternalInput")
    for i in (2, 3, 4):
        d[f"W{i}"] = nc.dram_tensor(f"W{i}", [FM, FM], F32, kind="ExternalInput")
    d["W5"] = nc.dram_tensor("W5", [FM, F5], F32, kind="ExternalInput")
    d["W6"] = nc.dram_tensor("W6", [F5, FO], F32, kind="ExternalInput")
    d["B14"] = nc.dram_tensor("B14", [P, 4 * KM], F32, kind="ExternalInput")
    d["b5rep"] = nc.dram_tensor("b5rep", [P, F5], F32, kind="ExternalInput")
    d["b6rep"] = nc.dram_tensor("b6rep", [P, FO], F32, kind="ExternalInput")
    d["gsrc"] = nc.dram_tensor("gsrc", [P, TC], I32, kind="ExternalInput")
    d["slot"] = nc.dram_tensor("slot", [P, TC], F32, kind="ExternalInput")
    d["normv"] = nc.dram_tensor("normv", [P, TC], F32, kind="ExternalInput")
    d["B14R"] = nc.dram_tensor("B14R", [P, 4 * FM], F32, kind="ExternalInput")
    d["dinv2"] = nc.dram_tensor("dinv2", [P, T], F32, kind="ExternalInput")
    out_d = nc.dram_tensor("out", [NP, FO], F32, kind="ExternalOutput")

    h512 = [nc.dram_tensor(f"h{i}", [NP, FM], MD, kind="Internal") for i in (1, 2, 3, 4)]
    h5_d = nc.dram_tensor("h5", [NP, F5], MD, kind="Internal")
    x6_d = nc.dram_tensor("x6", [NP, F5], MD, kind="Internal")

    Ident = mybir.ActivationFunctionType.Identity
    Relu = mybir.ActivationFunctionType.Relu

    with tile.TileContext(nc) as tc:
        with ExitStack() as ctx:
            res = ctx.enter_context(tc.tile_pool(name="res", bufs=1))
            gsrc_sb = res.tile([P, TC], I32)
            slot_sb = res.tile([P, TC], F32)
            norm_sb = res.tile([P, TC], F32)
            for name, t_sb in [("gsrc", gsrc_sb), ("slot", slot_sb), ("normv", norm_sb)]:
                nc.sync.dma_start(out=t_sb[:], in_=d[name][:, :])
            B14R_sb = res.tile([P, 4 * FM], F32)
            nc.sync.dma_start(out=B14R_sb[:], in_=d["B14R"][:, :])
            iota_i = res.tile([P, P], I32)
            nc.gpsimd.iota(iota_i[:], pattern=[[1, P]], base=0, channel_multiplier=0)
            iota_f = res.tile([P, P], F32)
            nc.vector.tensor_copy(out=iota_f[:], in_=iota_i[:])
            ident_sb = res.tile([P, P], F32)
            make_identity(nc, ident_sb[:])
            hcrep_sb = res.tile([P, FM], F32)
            nc.sync.dma_start(out=hcrep_sb[:], in_=d["hcrep"][:, :])
            B14_sb = res.tile([P, 4 * KM], F32)
            nc.sync.dma_start(out=B14_sb[:], in_=d["B14"][:, :])
            b5rep_sb = res.tile([P, F5], F32)
            nc.sync.dma_start(out=b5rep_sb[:], in_=d["b5rep"][:, :])
            b6rep_sb = res.tile([P, FO], F32)
            nc.sync.dma_start(out=b6rep_sb[:], in_=d["b6rep"][:, :])
            dinv2_sb = res.tile([P, T], F32)
            nc.sync.dma_start(out=dinv2_sb[:], in_=d["dinv2"][:, :])
            if MD is F32:
                iota_m, slot_m, norm_m = iota_f, slot_sb, norm_sb
            else:
                iota_m = res.tile([P, P], MD)
                nc.vector.tensor_copy(out=iota_m[:], in_=iota_f[:])
                slot_m = res.tile([P, TC], MD)
                nc.vector.tensor_copy(out=slot_m[:], in_=slot_sb[:])
                norm_m = res.tile([P, TC], MD)
                nc.vector.tensor_copy(out=norm_m[:], in_=norm_sb[:])


            # ---- layer 1 dense ----
            with tc.tile_pool(name="l1", bufs=1) as l1p, \
                 tc.tile_pool(name="l1ps", bufs=2, space="PSUM") as l1ps, \
                 tc.tile_pool(name="l1sb", bufs=3) as l1sb:
                xT1_sb = l1p.tile([3, NP], F32)
                nc.sync.dma_start(out=xT1_sb[:], in_=d["xT1"][:, :])
                W1v_sb = l1p.tile([3, FM], F32)
                nc.sync.dma_start(out=W1v_sb[:], in_=d["W1v"][:, :])
                for n in range(T):
                    ph = l1ps.tile([P, FM], F32, tag="ph")
                    nc.tensor.matmul(out=ph[:], lhsT=xT1_sb[:, n * P:(n + 1) * P],
                                     rhs=W1v_sb[:], start=True, stop=True)
                    hs = l1sb.tile([P, FM], MD, tag="hs")
                    nc.vector.tensor_add(out=hs[:], in0=ph[:], in1=hcrep_sb[:])
                    nc.sync.dma_start(out=h512[0][n * P:(n + 1) * P, :], in_=hs[:])

            def build_onehot(sp, t, c_cnt, slot_src, norm_src, tag):
                oh = sp.tile([P, c_cnt * P], MD, tag=tag, name=f"oh_{tag}_{t}")
                oh3 = oh[:].rearrange("p (c j) -> p c j", c=c_cnt)
                nc.vector.tensor_tensor(
                    out=oh3,
                    in0=slot_src[:, t * c_cnt:(t + 1) * c_cnt]
                        .rearrange("p (c u) -> p c u", u=1).to_broadcast([P, c_cnt, P]),
                    in1=iota_f[:].rearrange("p (u j) -> p u j", u=1)
                        .to_broadcast([P, c_cnt, P]),
                    op=mybir.AluOpType.is_equal,
                )
                nc.vector.tensor_tensor(
                    out=oh3, in0=oh3,
                    in1=norm_src[:, t * c_cnt:(t + 1) * c_cnt]
                        .rearrange("p (c u) -> p c u", u=1).to_broadcast([P, c_cnt, P]),
                    op=mybir.AluOpType.mult,
                )
                return oh

            # ---- merged phases: scatter(i) + dense(i+1), i = 1..4 ----
            # layer i scatter consumes h512[i-1]; dense(i+1) writes h512[i] or h5
            for i in (1, 2, 3, 4):
                relu = i in (2, 4)
                h_src = h512[i - 1]
                F_out = FM if i < 4 else F5
                h_dst = h512[i] if i < 4 else h5_d
                W_d = d[f"W{i + 1}"]
                with tc.tile_pool(name=f"ph{i}", bufs=int(__import__("os").environ.get("KBASS_BUFS", "2"))) as sp, \
                     tc.tile_pool(name=f"ph{i}w", bufs=1) as wp, \
                     tc.tile_pool(name=f"ph{i}ps", bufs=3, space="PSUM") as pp, \
                     tc.tile_pool(name=f"ph{i}pt", bufs=2, space="PSUM") as pt, \
                     tc.tile_pool(name=f"ph{i}pd", bufs=2, space="PSUM") as pd:
                    W_sb = [wp.tile([P, F_out], F32, tag=f"w{k}", name=f"w{i}_{k}")
                            for k in range(KM)]
                    for k in range(KM):
                        nc.sync.dma_start(out=W_sb[k][:], in_=W_d[k * P:(k + 1) * P, :])
                    for t in range(T):
                        # self-loop + bias term: diag-scaled h tile + replicated b_i
                        hre = sp.tile([P, FM], MD, tag="hre", name=f"hre{i}_{t}")
                        nc.sync.dma_start(out=hre[:], in_=h_src[t * P:(t + 1) * P, :])
                        sfb = sp.tile([P, FM], F32, tag="sfb", name=f"sfb{i}_{t}")
                        nc.vector.tensor_scalar_mul(
                            out=sfb[:], in0=hre[:], scalar1=dinv2_sb[:, t:t + 1])
                        nc.vector.tensor_add(
                            out=sfb[:], in0=sfb[:],
                            in1=B14R_sb[:, (i - 1) * FM:i * FM])
                        msg = sp.tile([P, C * FM], MD, tag="msg", name=f"msg{i}_{t}")
                        for c in range(C):
                            nc.gpsimd.indirect_dma_start(
                                out=msg[:, c * FM:(c + 1) * FM],
                                out_offset=None,
                                in_=h_src[:, :],
                                in_offset=bass.IndirectOffsetOnAxis(
                                    ap=gsrc_sb[:, t * C + c:t * C + c + 1], axis=0),
                            )
                        oh = build_onehot(sp, t, C, slot_m, norm_m, "oh")
                        # orientation A: node-major segment sum, onehot stationary
                        pa = pp.tile([P, FM], F32, tag="pa", name=f"pa{i}_{t}")
                        for c in range(C):
                            nc.tensor.matmul(
                                out=pa[:], lhsT=oh[:, c * P:(c + 1) * P],
                                rhs=msg[:, c * FM:(c + 1) * FM],
                                start=(c == 0), stop=(c == C - 1))
                        node = sp.tile([P, FM], F32, tag="node", name=f"nd{i}_{t}")
                        nc.vector.tensor_add(out=node[:], in0=pa[:], in1=sfb[:])
                        if relu:
                            nc.vector.tensor_scalar_max(out=node[:], in0=node[:],
                                                        scalar1=0.0)
                        # to feature-major via PE transposes
                        ptr = pt.tile([P, FM], F32, tag="ptr", name=f"pt{i}_{t}")
                        stage = sp.tile([P, FM], F32, tag="stage", name=f"st{i}_{t}")
                        for fo in range(KM):
                            nc.tensor.matmul(
                                out=ptr[:, fo * P:(fo + 1) * P],
                                lhsT=node[:, fo * P:(fo + 1) * P],
                                rhs=ident_sb[:], is_transpose=True,
                                start=True, stop=True)
                            nc.scalar.activation(
                                out=stage[:, fo * P:(fo + 1) * P],
                                in_=ptr[:, fo * P:(fo + 1) * P],
                                func=Ident, bias=0.0)
                        # dense(i+1) for this tile, straight from stage
                        ph = pd.tile([P, F_out], F32, tag="ph", name=f"pd{i}_{t}")
                        for k in range(KM):
                            nc.tensor.matmul(out=ph[:], lhsT=stage[:, k * P:(k + 1) * P],
                                             rhs=W_sb[k][:], start=(k == 0),
                                             stop=(k == KM - 1))
                        hs = sp.tile([P, F_out], MD, tag="hs", name=f"hs{i}_{t}")
                        nc.vector.tensor_copy(out=hs[:], in_=ph[:])
                        nc.sync.dma_start(out=h_dst[t * P:(t + 1) * P, :], in_=hs[:])

            # ---- layer 5 scatter (orientation A, self-loop via DVE add) ----
            with tc.tile_pool(name="s5", bufs=2) as sp5, \
                 tc.tile_pool(name="s5ps", bufs=2, space="PSUM") as pp5:
                for t in range(T):
                    hre5 = sp5.tile([P, F5], MD, tag="hre5", name=f"hr5_{t}")
                    nc.sync.dma_start(out=hre5[:], in_=h5_d[t * P:(t + 1) * P, :])
                    sfb5 = sp5.tile([P, F5], F32, tag="sfb5", name=f"sf5_{t}")
                    nc.vector.tensor_scalar_mul(
                        out=sfb5[:], in0=hre5[:], scalar1=dinv2_sb[:, t:t + 1])
                    nc.vector.tensor_add(out=sfb5[:], in0=sfb5[:], in1=b5rep_sb[:])
                    msg = sp5.tile([P, C * F5], MD, tag="msg5", name=f"m5_{t}")
                    for c in range(C):
                        nc.gpsimd.indirect_dma_start(
                            out=msg[:, c * F5:(c + 1) * F5],
                            out_offset=None,
                            in_=h5_d[:, :],
                            in_offset=bass.IndirectOffsetOnAxis(
                                ap=gsrc_sb[:, t * C + c:t * C + c + 1], axis=0),
                        )
                    oh = build_onehot(sp5, t, C, slot_m, norm_m, "oh5")
                    pa = pp5.tile([P, F5], F32, tag="pa", name=f"pa_{t}")
                    for c in range(C):
                        nc.tensor.matmul(out=pa[:], lhsT=oh[:, c * P:(c + 1) * P],
                                         rhs=msg[:, c * F5:(c + 1) * F5],
                                         start=(c == 0), stop=(c == C - 1))
                    xo = sp5.tile([P, F5], MD, tag="xo5", name=f"xo_{t}")
                    nc.vector.tensor_add(out=xo[:], in0=pa[:], in1=sfb5[:])
                    nc.sync.dma_start(out=x6_d[t * P:(t + 1) * P, :], in_=xo[:])

            # ---- layer 6: scatter (orientation B) + dense, interleaved ----
            with tc.tile_pool(name="s6", bufs=2) as sp6, \
                 tc.tile_pool(name="s6w", bufs=1) as wp6, \
                 tc.tile_pool(name="s6ps", bufs=2, space="PSUM") as pp6, \
                 tc.tile_pool(name="s6pd", bufs=2, space="PSUM") as pd6:
                W6_sb = wp6.tile([F5, FO], F32)
                nc.sync.dma_start(out=W6_sb[:], in_=d["W6"][:, :])
                for t in range(T):
                    hre = sp6.tile([P, F5], MD, tag="hre6", name=f"hre6_{t}")
                    nc.sync.dma_start(out=hre[:], in_=x6_d[t * P:(t + 1) * P, :])
                    hsc = sp6.tile([P, F5], F32, tag="hsc6", name=f"hsc6_{t}")
                    nc.vector.tensor_scalar_mul(
                        out=hsc[:], in0=hre[:], scalar1=dinv2_sb[:, t:t + 1])
                    msg = sp6.tile([P, C * F5], MD, tag="msg6", name=f"m6_{t}")
                    for c in range(C):
                        nc.gpsimd.indirect_dma_start(
                            out=msg[:, c * F5:(c + 1) * F5],
                            out_offset=None,
                            in_=x6_d[:, :],
                            in_offset=bass.IndirectOffsetOnAxis(
                                ap=gsrc_sb[:, t * C + c:t * C + c + 1], axis=0),
                        )
                    oh = build_onehot(sp6, t, C, slot_m, norm_m, "oh6")
                    pg = pp6.tile([F5, P], F32, tag="pg", name=f"pg_{t}")
                    nc.tensor.matmul(out=pg[:], lhsT=hsc[:], rhs=ident_sb[:],
                                     is_transpose=True, start=True, stop=False,
                                     skip_group_check=True)
                    for c in range(C):
                        nc.tensor.matmul(out=pg[:], lhsT=msg[:, c * F5:(c + 1) * F5],
                                         rhs=oh[:, c * P:(c + 1) * P],
                                         start=False, stop=(c == C - 1),
                                         skip_group_check=True)
                    gst = sp6.tile([F5, P], F32, tag="gst", name=f"g_{t}")
                    nc.scalar.activation(out=gst[:], in_=pg[:], func=Ident, bias=0.0)
                    pf = pd6.tile([P, FO], F32, tag="pf", name=f"pf_{t}")
                    nc.tensor.matmul(out=pf[:], lhsT=gst[:], rhs=W6_sb[:],
                                     start=True, stop=True)
                    os_ = sp6.tile([P, FO], F32, tag="os", name=f"o_{t}")
                    nc.vector.tensor_add(out=os_[:], in0=pf[:], in1=b6rep_sb[:])
                    nc.sync.dma_start(out=out_d[t * P:(t + 1) * P, :], in_=os_[:])

    _split_multiwaits(nc)
    return nc


def _prepare(batch_vertices, img_features, edge_indices,
             W1, b1, W2, b2, W3, b3, W4, b4, W5, b5, W6, b6):
    B, N, _ = batch_vertices.shape
    FM = W1.shape[1]
    F5 = W5.shape[1]
    FO = W6.shape[1]

    ei = np.asarray(edge_indices).astype(np.int64)
    g = _pack_graph(ei[0], ei[1], N)
    NP, T, C, perm = g["NP"], g["T"], g["C"], g["perm"]

    KM = FM // P
    hc = img_features.astype(np.float32) @ W1[3:].astype(np.float32)

    valid = perm >= 0
    vperm = np.zeros((B, NP, 3), np.float32)
    vperm[:, valid, :] = batch_vertices[:, perm[valid], :]

    common = {
        "W1v": np.ascontiguousarray(W1[:3].astype(np.float32)),
        "W2": np.ascontiguousarray(W2.astype(np.float32)),
        "W3": np.ascontiguousarray(W3.astype(np.float32)),
        "W4": np.ascontiguousarray(W4.astype(np.float32)),
        "W5": np.ascontiguousarray(W5.astype(np.float32)),
        "W6": np.ascontiguousarray(W6.astype(np.float32)),
        "B14": np.ascontiguousarray(
            np.stack([b.reshape(KM, P).T for b in (b1, b2, b3, b4)],
                     axis=1).reshape(P, 4 * KM).astype(np.float32)),
        "b5rep": np.tile(b5.astype(np.float32), (P, 1)),
        "b6rep": np.tile(b6.astype(np.float32), (P, 1)),
        "gsrc": g["gsrc"], "slot": g["slot"], "normv": g["norm"],
        "B14R": np.ascontiguousarray(
            np.tile(np.concatenate([b1, b2, b3, b4]).astype(np.float32), (P, 1))),
        "dinv2": g["dinv2"],
    }
    in_maps = []
    for b in range(B):
        m = dict(common)
        m["xT1"] = np.ascontiguousarray(vperm[b].T)
        m["hcrep"] = np.tile(hc[b], (P, 1))
        in_maps.append(m)
    meta = dict(NP=NP, T=T, C=C, perm=perm, valid=valid, B=B, N=N,
                FM=FM, F5=F5, FO=FO)
    return in_maps, meta


_BUILD_CACHE = {}


def run(inputs, trace=False):
    in_maps, meta = _prepare(**inputs)
    key = (meta["NP"], meta["C"], meta["FM"], meta["F5"], meta["FO"])
    if key not in _BUILD_CACHE:
        t0 = time.time()
        _BUILD_CACHE[key] = _build_nc(meta["NP"], meta["T"], meta["C"],
                                      meta["FM"], meta["F5"], meta["FO"])
        print(f"[kernel] built bass program in {time.time()-t0:.1f}s", file=sys.stderr)
    nc = _BUILD_CACHE[key]
    B = meta["B"]
    res = run_bass_kernel_spmd(nc, in_maps, core_ids=list(range(B)), trace=trace)
    perm, valid, N = meta["perm"], meta["valid"], meta["N"]
    out = np.empty((B, N, meta["FO"]), np.float32)
    for b in range(B):
        dev = res.results[b]["out"]
        out[b, perm[valid], :] = dev[valid, :]
    return out, res


def kernel(**inputs) -> np.ndarray:
    out, _ = run(inputs)
    return out


# revision 15
# speedup vs baseline: 1.0397x; 1.0397x over previous
"""GCN (6-layer GCNConv) Trainium2 Bass kernel — v2.

Data-parallel over batch (1 mesh per NeuronCore). Per layer
out = A_hat @ (x @ W) + b with A_hat = D^-1/2 (A+I) D^-1/2 shared across batch
and layers.

v2 structure (per core):
  - Host: symmetric-norm edge list WITHOUT self-loops (their contribution is
    added on-device as a PE transpose of the diag-scaled h tile, accumulated
    into the same PSUM segment-sum group). Nodes are relabeled (degree-balanced
    bin packing) so every 128-node dst tile has <= C*128 in-edges; edges are
    grouped per dst tile and padded to C chunks of 128.
  - Device: phases interleave scatter(i) with dense(i+1) per dst tile. The
    scatter's feature-major output tile (stage, SBUF) is consumed directly as
    the next dense matmul's lhsT — activations never round-trip through DRAM
    between layers; only the node-major h gather tables do.
  - Layer 1 uses the rank-1 structure of the broadcast image features:
    h1 = verts @ W1[:3] + (img @ W1[3:]) broadcast over nodes (host-computed).
  - Layer 5 scatter runs in orientation A (node-major out, self-loops kept as
    real edges) to produce the gather table for layer 6's message-first pass.
  - Layer 6: message passing first (64-wide), then the 64->3 matmul.
"""
import sys
import time

sys.path.insert(0, "/opt/trn_rl_repo")
import numpy as np
from contextlib import ExitStack

import concourse.bass as bass
import concourse.mybir as mybir
import concourse.tile as tile
from concourse.bass_utils import run_bass_kernel_spmd
from concourse.masks import make_identity

P = 128
F32 = mybir.dt.float32
I32 = mybir.dt.int32

_msw_ctr = [0]


def _split_multiwaits(nc, max_waits=1):
    """This walrus build rejects >1 sync wait per instruction: split extras
    onto preceding same-engine NOPs."""
    for f in nc.m.functions:
        for b in f.blocks:
            out, changed = [], False
            for inst in b.instructions:
                si = getattr(inst, "sync_info", None)
                waits = list(si.on_wait) if si is not None else []
                if len(waits) > max_waits:
                    changed = True
                    for w in waits[:-max_waits]:
                        _msw_ctr[0] += 1
                        nop = mybir.InstNoOp(name=f"msw-{_msw_ctr[0]}", ins=[], outs=[])
                        nop.engine = inst.engine
                        nop.sync_info = mybir.SyncInfo(on_wait=[w], on_update=[])
                        out.append(nop)
                    si.on_wait = waits[-max_waits:]
                out.append(inst)
            if changed:
                b.instructions = out
    return nc


def _pack_graph(src, dst, N):
    """Relabel nodes into degree-balanced 128-node tiles (no self-loops in the
    edge list). Returns device arrays [128, T*C] plus the with-self-loops
    variant [128, T*(C+1)] used by layer 5."""
    T = (N + P - 1) // P
    NP = T * P
    indeg = np.bincount(dst, minlength=N)          # no-loop in-degree
    C = max(1, int(np.ceil(len(src) / (T * P))))

    order = np.argsort(-indeg, kind="stable")
    while True:
        cap = C * P
        load = np.zeros(T, np.int64)
        count = np.zeros(T, np.int64)
        assign = np.empty(N, np.int64)
        ok = True
        for v in order:
            d = int(indeg[v])
            best_t, best_rem = -1, -1
            for t in range(T):
                if count[t] < P:
                    rem = cap - load[t]
                    if rem > best_rem:
                        best_rem, best_t = rem, t
            if best_t < 0 or load[best_t] + d > cap:
                ok = False
                break
            assign[v] = best_t
            load[best_t] += d
            count[best_t] += 1
        if ok:
            break
        C += 1

    perm = np.full(NP, -1, np.int64)
    new_of_old = np.empty(N, np.int64)
    cursor = np.zeros(T, np.int64)
    for v in range(N):
        t = assign[v]
        nid = t * P + cursor[t]
        cursor[t] += 1
        perm[nid] = v
        new_of_old[v] = nid

    # symmetric normalization (degree INCLUDES self-loops, per GCN)
    deg = (indeg + 1).astype(np.float32)
    dinv = (1.0 / np.sqrt(deg, dtype=np.float32)).astype(np.float32)
    norm = (dinv[src] * dinv[dst]).astype(np.float32)

    src_n = new_of_old[src]
    dst_n = new_of_old[dst]
    tile_of_e = dst_n // P
    order_e = np.argsort(tile_of_e, kind="stable")
    src_n, dst_n, norm = src_n[order_e], dst_n[order_e], norm[order_e]
    tile_of_e = tile_of_e[order_e]

    gsrc = np.zeros((T, C, P), np.int32)
    slot = np.zeros((T, C, P), np.float32)
    nrm = np.zeros((T, C, P), np.float32)
    starts = np.searchsorted(tile_of_e, np.arange(T + 1))
    for t in range(T):
        lo, hi = starts[t], starts[t + 1]
        n_e = hi - lo
        assert n_e <= C * P, (t, n_e, C * P)
        fs = np.zeros(C * P, np.int32)
        fl = np.zeros(C * P, np.float32)
        fn = np.zeros(C * P, np.float32)
        fs[:n_e] = src_n[lo:hi]
        fl[:n_e] = (dst_n[lo:hi] - t * P).astype(np.float32)
        fn[:n_e] = norm[lo:hi]
        gsrc[t] = fs.reshape(C, P)
        slot[t] = fl.reshape(C, P)
        nrm[t] = fn.reshape(C, P)

    # per-(slot, tile) dinv^2 for the on-device self-loop term (0 for dummies)
    dinv_new = np.zeros(NP, np.float32)
    valid = perm >= 0
    dinv_new[valid] = dinv[perm[valid]]
    dinv2 = (dinv_new ** 2).reshape(T, P).T.copy()   # [128, T]

    def dev(a):
        return np.ascontiguousarray(a.transpose(2, 0, 1).reshape(P, -1))

    return dict(NP=NP, T=T, C=C, perm=perm, dinv2=np.ascontiguousarray(dinv2),
                gsrc=dev(gsrc), slot=dev(slot), norm=dev(nrm))


def _build_nc(NP, T, C, FM, F5, FO):
    import os
    scratch = int(os.environ.get("KBASS_SCRATCH", "16384"))
    MD = mybir.dt.bfloat16 if os.environ.get("KBASS_MSGDT", "f32") == "bf16" else F32
    nc = bass.Bass(dynamic_dma_scratch_size=scratch)
    TC = T * C
    C5 = C + 1
    KM = FM // P
    K5 = FM // P

    d = {}
    d["xT1"] = nc.dram_tensor("xT1", [3, NP], F32, kind="ExternalInput")
    d["hcrep"] = nc.dram_tensor("hcrep", [P, FM], F32, kind="ExternalInput")
    d["W1v"] = nc.dram_tensor("W1v", [3, FM], F32, kind="ExternalInput")
    for i in (2, 3, 4):
        d[f"W{i}"] = nc.dram_tensor(f"W{i}", [FM, FM], F32, kind="ExternalInput")
    d["W5"] = nc.dram_tensor("W5", [FM, F5], F32, kind="ExternalInput")
    d["W6"] = nc.dram_tensor("W6", [F5, FO], F32, kind="ExternalInput")
    d["B14"] = nc.dram_tensor("B14", [P, 4 * KM], F32, kind="ExternalInput")
    d["b5rep"] = nc.dram_tensor("b5rep", [P, F5], F32, kind="ExternalInput")
    d["b6rep"] = nc.dram_tensor("b6rep", [P, FO], F32, kind="ExternalInput")
    d["gsrc"] = nc.dram_tensor("gsrc", [P, TC], I32, kind="ExternalInput")
    d["slot"] = nc.dram_tensor("slot", [P, TC], F32, kind="ExternalInput")
    d["normv"] = nc.dram_tensor("normv", [P, TC], F32, kind="ExternalInput")
    d["B14R"] = nc.dram_tensor("B14R", [P, 4 * FM], F32, kind="ExternalInput")
    d["dinv2"] = nc.dram_tensor("dinv2", [P, T], F32, kind="ExternalInput")
    out_d = nc.dram_tensor("out", [NP, FO], F32, kind="ExternalOutput")

    h512 = [nc.dram_tensor(f"h{i}", [NP, FM], MD, kind="Internal") for i in (1, 2, 3, 4)]
    h5_d = nc.dram_tensor("h5", [NP, F5], MD, kind="Internal")
    x6_d = nc.dram_tensor("x6", [NP, F5], MD, kind="Internal")

    Ident = mybir.ActivationFunctionType.Identity
    Relu = mybir.ActivationFunctionType.Relu

    with tile.TileContext(nc) as tc:
        with ExitStack() as ctx:
            res = ctx.enter_context(tc.tile_pool(name="res", bufs=1))
            gsrc_sb = res.tile([P, TC], I32)
            slot_sb = res.tile([P, TC], F32)
            norm_sb = res.tile([P, TC], F32)
            for name, t_sb in [("gsrc", gsrc_sb), ("slot", slot_sb), ("normv", norm_sb)]:
                nc.sync.dma_start(out=t_sb[:], in_=d[name][:, :])
            B14R_sb = res.tile([P, 4 * FM], F32)
            nc.sync.dma_start(out=B14R_sb[:], in_=d["B14R"][:, :])
            iota_i = res.tile([P, P], I32)
            nc.gpsimd.iota(iota_i[:], pattern=[[1, P]], base=0, channel_multiplier=0)
            iota_f = res.tile([P, P], F32)
            nc.vector.tensor_copy(out=iota_f[:], in_=iota_i[:])
            ident_sb = res.tile([P, P], F32)
            make_identity(nc, ident_sb[:])
            hcrep_sb = res.tile([P, FM], F32)
            nc.sync.dma_start(out=hcrep_sb[:], in_=d["hcrep"][:, :])
            B14_sb = res.tile([P, 4 * KM], F32)
            nc.sync.dma_start(out=B14_sb[:], in_=d["B14"][:, :])
            b5rep_sb = res.tile([P, F5], F32)
            nc.sync.dma_start(out=b5rep_sb[:], in_=d["b5rep"][:, :])
            b6rep_sb = res.tile([P, FO], F32)
            nc.sync.dma_start(out=b6rep_sb[:], in_=d["b6rep"][:, :])
            dinv2_sb = res.tile([P, T], F32)
            nc.sync.dma_start(out=dinv2_sb[:], in_=d["dinv2"][:, :])
            if MD is F32:
                iota_m, slot_m, norm_m = iota_f, slot_sb, norm_sb
            else:
                iota_m = res.tile([P, P], MD)
                nc.vector.tensor_copy(out=iota_m[:], in_=iota_f[:])
                slot_m = res.tile([P, TC], MD)
                nc.vector.tensor_copy(out=slot_m[:], in_=slot_sb[:])
                norm_m = res.tile([P, TC], MD)
                nc.vector.tensor_copy(out=norm_m[:], in_=norm_sb[:])


            # ---- layer 1 dense ----
            with tc.tile_pool(name="l1", bufs=1) as l1p, \
                 tc.tile_pool(name="l1ps", bufs=2, space="PSUM") as l1ps, \
                 tc.tile_pool(name="l1sb", bufs=3) as l1sb:
                xT1_sb = l1p.tile([3, NP], F32)
                nc.sync.dma_start(out=xT1_sb[:], in_=d["xT1"][:, :])
                W1v_sb = l1p.tile([3, FM], F32)
                nc.sync.dma_start(out=W1v_sb[:], in_=d["W1v"][:, :])
                for n in range(T):
                    ph = l1ps.tile([P, FM], F32, tag="ph")
                    nc.tensor.matmul(out=ph[:], lhsT=xT1_sb[:, n * P:(n + 1) * P],
                                     rhs=W1v_sb[:], start=True, stop=True)
                    hs = l1sb.tile([P, FM], MD, tag="hs")
                    nc.vector.tensor_add(out=hs[:], in0=ph[:], in1=hcrep_sb[:])
                    nc.sync.dma_start(out=h512[0][n * P:(n + 1) * P, :], in_=hs[:])

            def build_onehot(sp, t, c_cnt, slot_src, norm_src, tag):
                oh = sp.tile([P, c_cnt * P], MD, tag=tag, name=f"oh_{tag}_{t}")
                oh3 = oh[:].rearrange("p (c j) -> p c j", c=c_cnt)
                nc.vector.tensor_tensor(
                    out=oh3,
                    in0=slot_src[:, t * c_cnt:(t + 1) * c_cnt]
                        .rearrange("p (c u) -> p c u", u=1).to_broadcast([P, c_cnt, P]),
                    in1=iota_f[:].rearrange("p (u j) -> p u j", u=1)
                        .to_broadcast([P, c_cnt, P]),
                    op=mybir.AluOpType.is_equal,
                )
                nc.vector.tensor_tensor(
                    out=oh3, in0=oh3,
                    in1=norm_src[:, t * c_cnt:(t + 1) * c_cnt]
                        .rearrange("p (c u) -> p c u", u=1).to_broadcast([P, c_cnt, P]),
                    op=mybir.AluOpType.mult,
                )
                return oh

            # ---- merged phases: scatter(i) + dense(i+1), i = 1..4 ----
            # layer i scatter consumes h512[i-1]; dense(i+1) writes h512[i] or h5
            for i in (1, 2, 3, 4):
                relu = i in (2, 4)
                h_src = h512[i - 1]
                F_out = FM if i < 4 else F5
                h_dst = h512[i] if i < 4 else h5_d
                W_d = d[f"W{i + 1}"]
                with tc.tile_pool(name=f"ph{i}", bufs=int(__import__("os").environ.get("KBASS_BUFS", "2"))) as sp, \
                     tc.tile_pool(name=f"ph{i}w", bufs=1) as wp, \
                     tc.tile_pool(name=f"ph{i}ps", bufs=2, space="PSUM") as pp, \
                     tc.tile_pool(name=f"ph{i}pt", bufs=2, space="PSUM") as pt, \
                     tc.tile_pool(name=f"ph{i}pd", bufs=2, space="PSUM") as pd:
                    W_sb = [wp.tile([P, F_out], F32, tag=f"w{k}", name=f"w{i}_{k}")
                            for k in range(KM)]
                    for k in range(KM):
                        nc.sync.dma_start(out=W_sb[k][:], in_=W_d[k * P:(k + 1) * P, :])
                    for t in range(T):
                        # self-loop + bias term: diag-scaled h tile + replicated b_i
                        hre = sp.tile([P, FM], MD, tag="hre", name=f"hre{i}_{t}")
                        nc.sync.dma_start(out=hre[:], in_=h_src[t * P:(t + 1) * P, :])
                        sfb = sp.tile([P, FM], F32, tag="sfb", name=f"sfb{i}_{t}")
                        nc.vector.tensor_scalar_mul(
                            out=sfb[:], in0=hre[:], scalar1=dinv2_sb[:, t:t + 1])
                        nc.vector.tensor_add(
                            out=sfb[:], in0=sfb[:],
                            in1=B14R_sb[:, (i - 1) * FM:i * FM])
                        msg = sp.tile([P, C * FM], MD, tag="msg", name=f"msg{i}_{t}")
                        for c in range(C):
                            nc.gpsimd.indirect_dma_start(
                                out=msg[:, c * FM:(c + 1) * FM],
                                out_offset=None,
                                in_=h_src[:, :],
                                in_offset=bass.IndirectOffsetOnAxis(
                                    ap=gsrc_sb[:, t * C + c:t * C + c + 1], axis=0),
                            )
                        oh = build_onehot(sp, t, C, slot_m, norm_m, "oh")
                        # orientation A: node-major segment sum, onehot stationary
                        pa = pp.tile([P, FM], F32, tag="pa", name=f"pa{i}_{t}")
                        for c in range(C):
                            nc.tensor.matmul(
                                out=pa[:], lhsT=oh[:, c * P:(c + 1) * P],
                                rhs=msg[:, c * FM:(c + 1) * FM],
                                start=(c == 0), stop=(c == C - 1))
                        node = sp.tile([P, FM], F32, tag="node", name=f"nd{i}_{t}")
                        nc.vector.tensor_add(out=node[:], in0=pa[:], in1=sfb[:])
                        if relu:
                            nc.vector.tensor_scalar_max(out=node[:], in0=node[:],
                                                        scalar1=0.0)
                        # to feature-major via PE transposes
                        ptr = pt.tile([P, FM], F32, tag="ptr", name=f"pt{i}_{t}")
                        stage = sp.tile([P, FM], F32, tag="stage", name=f"st{i}_{t}")
                        for fo in range(KM):
                            nc.tensor.matmul(
                                out=ptr[:, fo * P:(fo + 1) * P],
                                lhsT=node[:, fo * P:(fo + 1) * P],
                                rhs=ident_sb[:], is_transpose=True,
                                start=True, stop=True)
                            nc.scalar.activation(
                                out=stage[:, fo * P:(fo + 1) * P],
                                in_=ptr[:, fo * P:(fo + 1) * P],
                                func=Ident, bias=0.0)
                        # dense(i+1) for this tile, straight from stage
                        ph = pd.tile([P, F_out], F32, tag="ph", name=f"pd{i}_{t}")
                        for k in range(KM):
                            nc.tensor.matmul(out=ph[:], lhsT=stage[:, k * P:(k + 1) * P],
                                             rhs=W_sb[k][:], start=(k == 0),
                                             stop=(k == KM - 1))
                        hs = sp.tile([P, F_out], MD, tag="hs", name=f"hs{i}_{t}")
                        nc.vector.tensor_copy(out=hs[:], in_=ph[:])
                        nc.sync.dma_start(out=h_dst[t * P:(t + 1) * P, :], in_=hs[:])

            # ---- layer 5 scatter (orientation A, self-loop via DVE add) ----
            with tc.tile_pool(name="s5", bufs=2) as sp5, \
                 tc.tile_pool(name="s5ps", bufs=2, space="PSUM") as pp5:
                for t in range(T):
                    hre5 = sp5.tile([P, F5], MD, tag="hre5", name=f"hr5_{t}")
                    nc.sync.dma_start(out=hre5[:], in_=h5_d[t * P:(t + 1) * P, :])
                    sfb5 = sp5.tile([P, F5], F32, tag="sfb5", name=f"sf5_{t}")
                    nc.vector.tensor_scalar_mul(
                        out=sfb5[:], in0=hre5[:], scalar1=dinv2_sb[:, t:t + 1])
                    nc.vector.tensor_add(out=sfb5[:], in0=sfb5[:], in1=b5rep_sb[:])
                    msg = sp5.tile([P, C * F5], MD, tag="msg5", name=f"m5_{t}")
                    for c in range(C):
                        nc.gpsimd.indirect_dma_start(
                            out=msg[:, c * F5:(c + 1) * F5],
                            out_offset=None,
                            in_=h5_d[:, :],
                            in_offset=bass.IndirectOffsetOnAxis(
                                ap=gsrc_sb[:, t * C + c:t * C + c + 1], axis=0),
                        )
                    oh = build_onehot(sp5, t, C, slot_m, norm_m, "oh5")
                    pa = pp5.tile([P, F5], F32, tag="pa", name=f"pa_{t}")
                    for c in range(C):
                        nc.tensor.matmul(out=pa[:], lhsT=oh[:, c * P:(c + 1) * P],
                                         rhs=msg[:, c * F5:(c + 1) * F5],
                                         start=(c == 0), stop=(c == C - 1))
                    xo = sp5.tile([P, F5], MD, tag="xo5", name=f"xo_{t}")
                    nc.vector.tensor_add(out=xo[:], in0=pa[:], in1=sfb5[:])
                    nc.sync.dma_start(out=x6_d[t * P:(t + 1) * P, :], in_=xo[:])

            # ---- layer 6: scatter (orientation B) + dense, interleaved ----
            with tc.tile_pool(name="s6", bufs=2) as sp6, \
                 tc.tile_pool(name="s6w", bufs=1) as wp6, \
                 tc.tile_pool(name="s6ps", bufs=2, space="PSUM") as pp6, \
                 tc.tile_pool(name="s6pd", bufs=2, space="PSUM") as pd6:
                W6_sb = wp6.tile([F5, FO], F32)
                nc.sync.dma_start(out=W6_sb[:], in_=d["W6"][:, :])
                for t in range(T):
                    hre = sp6.tile([P, F5], MD, tag="hre6", name=f"hre6_{t}")
                    nc.sync.dma_start(out=hre[:], in_=x6_d[t * P:(t + 1) * P, :])
                    hsc = sp6.tile([P, F5], F32, tag="hsc6", name=f"hsc6_{t}")
                    nc.vector.tensor_scalar_mul(
                        out=hsc[:], in0=hre[:], scalar1=dinv2_sb[:, t:t + 1])
                    msg = sp6.tile([P, C * F5], MD, tag="msg6", name=f"m6_{t}")
                    for c in range(C):
                        nc.gpsimd.indirect_dma_start(
                            out=msg[:, c * F5:(c + 1) * F5],
                            out_offset=None,
                            in_=x6_d[:, :],
                            in_offset=bass.IndirectOffsetOnAxis(
                                ap=gsrc_sb[:, t * C + c:t * C + c + 1], axis=0),
                        )
                    oh = build_onehot(sp6, t, C, slot_m, norm_m, "oh6")
                    pg = pp6.tile([F5, P], F32, tag="pg", name=f"pg_{t}")
                    nc.tensor.matmul(out=pg[:], lhsT=hsc[:], rhs=ident_sb[:],
                                     is_transpose=True, start=True, stop=False,
                                     skip_group_check=True)
                    for c in range(C):
                        nc.tensor.matmul(out=pg[:], lhsT=msg[:, c * F5:(c + 1) * F5],
                                         rhs=oh[:, c * P:(c + 1) * P],
                                         start=False, stop=(c == C - 1),
                                         skip_group_check=True)
                    gst = sp6.tile([F5, P], F32, tag="gst", name=f"g_{t}")
                    nc.scalar.activation(out=gst[:], in_=pg[:], func=Ident, bias=0.0)
                    pf = pd6.tile([P, FO], F32, tag="pf", name=f"pf_{t}")
                    nc.tensor.matmul(out=pf[:], lhsT=gst[:], rhs=W6_sb[:],
                                     start=True, stop=True)
                    os_ = sp6.tile([P, FO], F32, tag="os", name=f"o_{t}")
                    nc.vector.tensor_add(out=os_[:], in0=pf[:], in1=b6rep_sb[:])
                    nc.sync.dma_start(out=out_d[t * P:(t + 1) * P, :], in_=os_[:])

    _split_multiwaits(nc)
    return nc


def _prepare(batch_vertices, img_features, edge_indices,
             W1, b1, W2, b2, W3, b3, W4, b4, W5, b5, W6, b6):
    B, N, _ = batch_vertices.shape
    FM = W1.shape[1]
    F5 = W5.shape[1]
    FO = W6.shape[1]

    ei = np.asarray(edge_indices).astype(np.int64)
    g = _pack_graph(ei[0], ei[1], N)
    NP, T, C, perm = g["NP"], g["T"], g["C"], g["perm"]

    KM = FM // P
    hc = img_features.astype(np.float32) @ W1[3:].astype(np.float32)

    valid = perm >= 0
    vperm = np.zeros((B, NP, 3), np.float32)
    vperm[:, valid, :] = batch_vertices[:, perm[valid], :]

    common = {
        "W1v": np.ascontiguousarray(W1[:3].astype(np.float32)),
        "W2": np.ascontiguousarray(W2.astype(np.float32)),
        "W3": np.ascontiguousarray(W3.astype(np.float32)),
        "W4": np.ascontiguousarray(W4.astype(np.float32)),
        "W5": np.ascontiguousarray(W5.astype(np.float32)),
        "W6": np.ascontiguousarray(W6.astype(np.float32)),
        "B14": np.ascontiguousarray(
            np.stack([b.reshape(KM, P).T for b in (b1, b2, b3, b4)],
                     axis=1).reshape(P, 4 * KM).astype(np.float32)),
        "b5rep": np.tile(b5.astype(np.float32), (P, 1)),
        "b6rep": np.tile(b6.astype(np.float32), (P, 1)),
        "gsrc": g["gsrc"], "slot": g["slot"], "normv": g["norm"],
        "B14R": np.ascontiguousarray(
            np.tile(np.concatenate([b1, b2, b3, b4]).astype(np.float32), (P, 1))),
        "dinv2": g["dinv2"],
    }
    in_maps = []
    for b in range(B):
        m = dict(common)
        m["xT1"] = np.ascontiguousarray(vperm[b].T)
        m["hcrep"] = np.tile(hc[b], (P, 1))
        in_maps.append(m)
    meta = dict(NP=NP, T=T, C=C, perm=perm, valid=valid, B=B, N=N,
                FM=FM, F5=F5, FO=FO)
    return in_maps, meta


_BUILD_CACHE = {}


def run(inputs, trace=False):
    in_maps, meta = _prepare(**inputs)
    key = (meta["NP"], meta["C"], meta["FM"], meta["F5"], meta["FO"])
    if key not in _BUILD_CACHE:
        t0 = time.time()
        _BUILD_CACHE[key] = _build_nc(meta["NP"], meta["T"], meta["C"],
                                      meta["FM"], meta["F5"], meta["FO"])
        print(f"[kernel] built bass program in {time.time()-t0:.1f}s", file=sys.stderr)
    nc = _BUILD_CACHE[key]
    B = meta["B"]
    res = run_bass_kernel_spmd(nc, in_maps, core_ids=list(range(B)), trace=trace)
    perm, valid, N = meta["perm"], meta["valid"], meta["N"]
    out = np.empty((B, N, meta["FO"]), np.float32)
    for b in range(B):
        dev = res.results[b]["out"]
        out[b, perm[valid], :] = dev[valid, :]
    return out, res


def kernel(**inputs) -> np.ndarray:
    out, _ = run(inputs)
    return out


# revision 16
# speedup vs baseline: 1.0454x; 1.0055x over previous
"""GCN (6-layer GCNConv) Trainium2 Bass kernel — v2.

Data-parallel over batch (1 mesh per NeuronCore). Per layer
out = A_hat @ (x @ W) + b with A_hat = D^-1/2 (A+I) D^-1/2 shared across batch
and layers.

v2 structure (per core):
  - Host: symmetric-norm edge list WITHOUT self-loops (their contribution is
    added on-device as a PE transpose of the diag-scaled h tile, accumulated
    into the same PSUM segment-sum group). Nodes are relabeled (degree-balanced
    bin packing) so every 128-node dst tile has <= C*128 in-edges; edges are
    grouped per dst tile and padded to C chunks of 128.
  - Device: phases interleave scatter(i) with dense(i+1) per dst tile. The
    scatter's feature-major output tile (stage, SBUF) is consumed directly as
    the next dense matmul's lhsT — activations never round-trip through DRAM
    between layers; only the node-major h gather tables do.
  - Layer 1 uses the rank-1 structure of the broadcast image features:
    h1 = verts @ W1[:3] + (img @ W1[3:]) broadcast over nodes (host-computed).
  - Layer 5 scatter runs in orientation A (node-major out, self-loops kept as
    real edges) to produce the gather table for layer 6's message-first pass.
  - Layer 6: message passing first (64-wide), then the 64->3 matmul.
"""
import sys
import time

sys.path.insert(0, "/opt/trn_rl_repo")
import numpy as np
from contextlib import ExitStack

import concourse.bass as bass
import concourse.mybir as mybir
import concourse.tile as tile
from concourse.bass_utils import run_bass_kernel_spmd
from concourse.masks import make_identity

P = 128
F32 = mybir.dt.float32
I32 = mybir.dt.int32

_msw_ctr = [0]


def _split_multiwaits(nc, max_waits=1):
    """This walrus build rejects >1 sync wait per instruction: split extras
    onto preceding same-engine NOPs."""
    for f in nc.m.functions:
        for b in f.blocks:
            out, changed = [], False
            for inst in b.instructions:
                si = getattr(inst, "sync_info", None)
                waits = list(si.on_wait) if si is not None else []
                if len(waits) > max_waits:
                    changed = True
                    for w in waits[:-max_waits]:
                        _msw_ctr[0] += 1
                        nop = mybir.InstNoOp(name=f"msw-{_msw_ctr[0]}", ins=[], outs=[])
                        nop.engine = inst.engine
                        nop.sync_info = mybir.SyncInfo(on_wait=[w], on_update=[])
                        out.append(nop)
                    si.on_wait = waits[-max_waits:]
                out.append(inst)
            if changed:
                b.instructions = out
    return nc


def _pack_graph(src, dst, N):
    """Relabel nodes into degree-balanced 128-node tiles (no self-loops in the
    edge list). Returns device arrays [128, T*C] plus the with-self-loops
    variant [128, T*(C+1)] used by layer 5."""
    T = (N + P - 1) // P
    NP = T * P
    indeg = np.bincount(dst, minlength=N)          # no-loop in-degree
    C = max(1, int(np.ceil(len(src) / (T * P))))

    order = np.argsort(-indeg, kind="stable")
    while True:
        cap = C * P
        load = np.zeros(T, np.int64)
        count = np.zeros(T, np.int64)
        assign = np.empty(N, np.int64)
        ok = True
        for v in order:
            d = int(indeg[v])
            best_t, best_rem = -1, -1
            for t in range(T):
                if count[t] < P:
                    rem = cap - load[t]
                    if rem > best_rem:
                        best_rem, best_t = rem, t
            if best_t < 0 or load[best_t] + d > cap:
                ok = False
                break
            assign[v] = best_t
            load[best_t] += d
            count[best_t] += 1
        if ok:
            break
        C += 1

    perm = np.full(NP, -1, np.int64)
    new_of_old = np.empty(N, np.int64)
    cursor = np.zeros(T, np.int64)
    for v in range(N):
        t = assign[v]
        nid = t * P + cursor[t]
        cursor[t] += 1
        perm[nid] = v
        new_of_old[v] = nid

    # symmetric normalization (degree INCLUDES self-loops, per GCN)
    deg = (indeg + 1).astype(np.float32)
    dinv = (1.0 / np.sqrt(deg, dtype=np.float32)).astype(np.float32)
    norm = (dinv[src] * dinv[dst]).astype(np.float32)

    src_n = new_of_old[src]
    dst_n = new_of_old[dst]
    tile_of_e = dst_n // P
    order_e = np.argsort(tile_of_e, kind="stable")
    src_n, dst_n, norm = src_n[order_e], dst_n[order_e], norm[order_e]
    tile_of_e = tile_of_e[order_e]

    gsrc = np.zeros((T, C, P), np.int32)
    slot = np.zeros((T, C, P), np.float32)
    nrm = np.zeros((T, C, P), np.float32)
    starts = np.searchsorted(tile_of_e, np.arange(T + 1))
    for t in range(T):
        lo, hi = starts[t], starts[t + 1]
        n_e = hi - lo
        assert n_e <= C * P, (t, n_e, C * P)
        fs = np.zeros(C * P, np.int32)
        fl = np.zeros(C * P, np.float32)
        fn = np.zeros(C * P, np.float32)
        fs[:n_e] = src_n[lo:hi]
        fl[:n_e] = (dst_n[lo:hi] - t * P).astype(np.float32)
        fn[:n_e] = norm[lo:hi]
        gsrc[t] = fs.reshape(C, P)
        slot[t] = fl.reshape(C, P)
        nrm[t] = fn.reshape(C, P)

    # per-(slot, tile) dinv^2 for the on-device self-loop term (0 for dummies)
    dinv_new = np.zeros(NP, np.float32)
    valid = perm >= 0
    dinv_new[valid] = dinv[perm[valid]]
    dinv2 = (dinv_new ** 2).reshape(T, P).T.copy()   # [128, T]

    def dev(a):
        return np.ascontiguousarray(a.transpose(2, 0, 1).reshape(P, -1))

    return dict(NP=NP, T=T, C=C, perm=perm, dinv2=np.ascontiguousarray(dinv2),
                gsrc=dev(gsrc), slot=dev(slot), norm=dev(nrm))


def _build_nc(NP, T, C, FM, F5, FO):
    import os
    scratch = int(os.environ.get("KBASS_SCRATCH", "16384"))
    MD = mybir.dt.bfloat16 if os.environ.get("KBASS_MSGDT", "f32") == "bf16" else F32
    nc = bass.Bass(dynamic_dma_scratch_size=scratch)
    TC = T * C
    C5 = C + 1
    KM = FM // P
    K5 = FM // P

    d = {}
    d["xT1"] = nc.dram_tensor("xT1", [3, NP], F32, kind="ExternalInput")
    d["hcrep"] = nc.dram_tensor("hcrep", [P, FM], F32, kind="ExternalInput")
    d["W1v"] = nc.dram_tensor("W1v", [3, FM], F32, kind="ExternalInput")
    for i in (2, 3, 4):
        d[f"W{i}"] = nc.dram_tensor(f"W{i}", [FM, FM], F32, kind="ExternalInput")
    d["W5"] = nc.dram_tensor("W5", [FM, F5], F32, kind="ExternalInput")
    d["W6"] = nc.dram_tensor("W6", [F5, FO], F32, kind="ExternalInput")
    d["B14"] = nc.dram_tensor("B14", [P, 4 * KM], F32, kind="ExternalInput")
    d["b5rep"] = nc.dram_tensor("b5rep", [P, F5], F32, kind="ExternalInput")
    d["b6rep"] = nc.dram_tensor("b6rep", [P, FO], F32, kind="ExternalInput")
    d["gsrc"] = nc.dram_tensor("gsrc", [P, TC], I32, kind="ExternalInput")
    d["slot"] = nc.dram_tensor("slot", [P, TC], F32, kind="ExternalInput")
    d["normv"] = nc.dram_tensor("normv", [P, TC], F32, kind="ExternalInput")
    d["B14R"] = nc.dram_tensor("B14R", [P, 4 * FM], F32, kind="ExternalInput")
    d["dinv2"] = nc.dram_tensor("dinv2", [P, T], F32, kind="ExternalInput")
    out_d = nc.dram_tensor("out", [NP, FO], F32, kind="ExternalOutput")

    h512 = [nc.dram_tensor(f"h{i}", [NP, FM], MD, kind="Internal") for i in (1, 2, 3, 4)]
    h5_d = nc.dram_tensor("h5", [NP, F5], MD, kind="Internal")
    x6_d = nc.dram_tensor("x6", [NP, F5], MD, kind="Internal")

    Ident = mybir.ActivationFunctionType.Identity
    Relu = mybir.ActivationFunctionType.Relu

    with tile.TileContext(nc) as tc:
        with ExitStack() as ctx:
            res = ctx.enter_context(tc.tile_pool(name="res", bufs=1))
            gsrc_sb = res.tile([P, TC], I32)
            slot_sb = res.tile([P, TC], F32)
            norm_sb = res.tile([P, TC], F32)
            for name, t_sb in [("gsrc", gsrc_sb), ("slot", slot_sb), ("normv", norm_sb)]:
                nc.sync.dma_start(out=t_sb[:], in_=d[name][:, :])
            B14R_sb = res.tile([P, 4 * FM], F32)
            nc.sync.dma_start(out=B14R_sb[:], in_=d["B14R"][:, :])
            iota_i = res.tile([P, P], I32)
            nc.gpsimd.iota(iota_i[:], pattern=[[1, P]], base=0, channel_multiplier=0)
            iota_f = res.tile([P, P], F32)
            nc.vector.tensor_copy(out=iota_f[:], in_=iota_i[:])
            ident_sb = res.tile([P, P], F32)
            make_identity(nc, ident_sb[:])
            hcrep_sb = res.tile([P, FM], F32)
            nc.sync.dma_start(out=hcrep_sb[:], in_=d["hcrep"][:, :])
            B14_sb = res.tile([P, 4 * KM], F32)
            nc.sync.dma_start(out=B14_sb[:], in_=d["B14"][:, :])
            b5rep_sb = res.tile([P, F5], F32)
            nc.sync.dma_start(out=b5rep_sb[:], in_=d["b5rep"][:, :])
            b6rep_sb = res.tile([P, FO], F32)
            nc.sync.dma_start(out=b6rep_sb[:], in_=d["b6rep"][:, :])
            dinv2_sb = res.tile([P, T], F32)
            nc.sync.dma_start(out=dinv2_sb[:], in_=d["dinv2"][:, :])
            if MD is F32:
                iota_m, slot_m, norm_m = iota_f, slot_sb, norm_sb
            else:
                iota_m = res.tile([P, P], MD)
                nc.vector.tensor_copy(out=iota_m[:], in_=iota_f[:])
                slot_m = res.tile([P, TC], MD)
                nc.vector.tensor_copy(out=slot_m[:], in_=slot_sb[:])
                norm_m = res.tile([P, TC], MD)
                nc.vector.tensor_copy(out=norm_m[:], in_=norm_sb[:])


            # ---- layer 1 dense ----
            with tc.tile_pool(name="l1", bufs=1) as l1p, \
                 tc.tile_pool(name="l1ps", bufs=2, space="PSUM") as l1ps, \
                 tc.tile_pool(name="l1sb", bufs=3) as l1sb:
                xT1_sb = l1p.tile([3, NP], F32)
                nc.sync.dma_start(out=xT1_sb[:], in_=d["xT1"][:, :])
                W1v_sb = l1p.tile([3, FM], F32)
                nc.sync.dma_start(out=W1v_sb[:], in_=d["W1v"][:, :])
                for n in range(T):
                    ph = l1ps.tile([P, FM], F32, tag="ph")
                    nc.tensor.matmul(out=ph[:], lhsT=xT1_sb[:, n * P:(n + 1) * P],
                                     rhs=W1v_sb[:], start=True, stop=True)
                    hs = l1sb.tile([P, FM], MD, tag="hs")
                    nc.vector.tensor_add(out=hs[:], in0=ph[:], in1=hcrep_sb[:])
                    nc.sync.dma_start(out=h512[0][n * P:(n + 1) * P, :], in_=hs[:])

            def build_onehot(sp, t, c_cnt, slot_src, norm_src, tag):
                oh = sp.tile([P, c_cnt * P], MD, tag=tag, name=f"oh_{tag}_{t}")
                oh3 = oh[:].rearrange("p (c j) -> p c j", c=c_cnt)
                nc.vector.tensor_tensor(
                    out=oh3,
                    in0=slot_src[:, t * c_cnt:(t + 1) * c_cnt]
                        .rearrange("p (c u) -> p c u", u=1).to_broadcast([P, c_cnt, P]),
                    in1=iota_f[:].rearrange("p (u j) -> p u j", u=1)
                        .to_broadcast([P, c_cnt, P]),
                    op=mybir.AluOpType.is_equal,
                )
                nc.vector.tensor_tensor(
                    out=oh3, in0=oh3,
                    in1=norm_src[:, t * c_cnt:(t + 1) * c_cnt]
                        .rearrange("p (c u) -> p c u", u=1).to_broadcast([P, c_cnt, P]),
                    op=mybir.AluOpType.mult,
                )
                return oh

            # ---- merged phases: scatter(i) + dense(i+1), i = 1..4 ----
            # layer i scatter consumes h512[i-1]; dense(i+1) writes h512[i] or h5
            for i in (1, 2, 3, 4):
                relu = i in (2, 4)
                h_src = h512[i - 1]
                F_out = FM if i < 4 else F5
                h_dst = h512[i] if i < 4 else h5_d
                W_d = d[f"W{i + 1}"]
                with tc.tile_pool(name=f"ph{i}", bufs=int(__import__("os").environ.get("KBASS_BUFS", "2"))) as sp, \
                     tc.tile_pool(name=f"ph{i}w", bufs=1) as wp, \
                     tc.tile_pool(name=f"ph{i}ps", bufs=2, space="PSUM") as pp, \
                     tc.tile_pool(name=f"ph{i}pt", bufs=2, space="PSUM") as pt, \
                     tc.tile_pool(name=f"ph{i}pd", bufs=2, space="PSUM") as pd:
                    W_sb = [wp.tile([P, F_out], F32, tag=f"w{k}", name=f"w{i}_{k}")
                            for k in range(KM)]
                    for k in range(KM):
                        nc.sync.dma_start(out=W_sb[k][:], in_=W_d[k * P:(k + 1) * P, :])
                    for t in range(T):
                        # self-loop + bias term: diag-scaled h tile + replicated b_i
                        hre = sp.tile([P, FM], MD, tag="hre", name=f"hre{i}_{t}")
                        nc.sync.dma_start(out=hre[:], in_=h_src[t * P:(t + 1) * P, :])
                        sfb = sp.tile([P, FM], F32, tag="sfb", name=f"sfb{i}_{t}")
                        nc.vector.tensor_scalar_mul(
                            out=sfb[:], in0=hre[:], scalar1=dinv2_sb[:, t:t + 1])
                        nc.vector.tensor_add(
                            out=sfb[:], in0=sfb[:],
                            in1=B14R_sb[:, (i - 1) * FM:i * FM])
                        msgs = []
                        for c in range(C):
                            mc = sp.tile([P, FM], MD, tag=f"msg{c}", name=f"msg{i}_{t}_{c}")
                            nc.gpsimd.indirect_dma_start(
                                out=mc[:],
                                out_offset=None,
                                in_=h_src[:, :],
                                in_offset=bass.IndirectOffsetOnAxis(
                                    ap=gsrc_sb[:, t * C + c:t * C + c + 1], axis=0),
                            )
                            msgs.append(mc)
                        oh = build_onehot(sp, t, C, slot_m, norm_m, "oh")
                        # orientation A: node-major segment sum, onehot stationary
                        pa = pp.tile([P, FM], F32, tag="pa", name=f"pa{i}_{t}")
                        for c in range(C):
                            nc.tensor.matmul(
                                out=pa[:], lhsT=oh[:, c * P:(c + 1) * P],
                                rhs=msgs[c][:],
                                start=(c == 0), stop=(c == C - 1))
                        node = sp.tile([P, FM], F32, tag="node", name=f"nd{i}_{t}")
                        nc.vector.tensor_add(out=node[:], in0=pa[:], in1=sfb[:])
                        if relu:
                            nc.vector.tensor_scalar_max(out=node[:], in0=node[:],
                                                        scalar1=0.0)
                        # to feature-major via PE transposes
                        ptr = pt.tile([P, FM], F32, tag="ptr", name=f"pt{i}_{t}")
                        stage = sp.tile([P, FM], F32, tag="stage", name=f"st{i}_{t}")
                        for fo in range(KM):
                            nc.tensor.matmul(
                                out=ptr[:, fo * P:(fo + 1) * P],
                                lhsT=node[:, fo * P:(fo + 1) * P],
                                rhs=ident_sb[:], is_transpose=True,
                                start=True, stop=True)
                            nc.scalar.activation(
                                out=stage[:, fo * P:(fo + 1) * P],
                                in_=ptr[:, fo * P:(fo + 1) * P],
                                func=Ident, bias=0.0)
                        # dense(i+1) for this tile, straight from stage
                        ph = pd.tile([P, F_out], F32, tag="ph", name=f"pd{i}_{t}")
                        for k in range(KM):
                            nc.tensor.matmul(out=ph[:], lhsT=stage[:, k * P:(k + 1) * P],
                                             rhs=W_sb[k][:], start=(k == 0),
                                             stop=(k == KM - 1))
                        hs = sp.tile([P, F_out], MD, tag="hs", name=f"hs{i}_{t}")
                        nc.vector.tensor_copy(out=hs[:], in_=ph[:])
                        nc.sync.dma_start(out=h_dst[t * P:(t + 1) * P, :], in_=hs[:])

            # ---- layer 5 scatter (orientation A, self-loop via DVE add) ----
            with tc.tile_pool(name="s5", bufs=2) as sp5, \
                 tc.tile_pool(name="s5ps", bufs=2, space="PSUM") as pp5:
                for t in range(T):
                    hre5 = sp5.tile([P, F5], MD, tag="hre5", name=f"hr5_{t}")
                    nc.sync.dma_start(out=hre5[:], in_=h5_d[t * P:(t + 1) * P, :])
                    sfb5 = sp5.tile([P, F5], F32, tag="sfb5", name=f"sf5_{t}")
                    nc.vector.tensor_scalar_mul(
                        out=sfb5[:], in0=hre5[:], scalar1=dinv2_sb[:, t:t + 1])
                    nc.vector.tensor_add(out=sfb5[:], in0=sfb5[:], in1=b5rep_sb[:])
                    msg = sp5.tile([P, C * F5], MD, tag="msg5", name=f"m5_{t}")
                    for c in range(C):
                        nc.gpsimd.indirect_dma_start(
                            out=msg[:, c * F5:(c + 1) * F5],
                            out_offset=None,
                            in_=h5_d[:, :],
                            in_offset=bass.IndirectOffsetOnAxis(
                                ap=gsrc_sb[:, t * C + c:t * C + c + 1], axis=0),
                        )
                    oh = build_onehot(sp5, t, C, slot_m, norm_m, "oh5")
                    pa = pp5.tile([P, F5], F32, tag="pa", name=f"pa_{t}")
                    for c in range(C):
                        nc.tensor.matmul(out=pa[:], lhsT=oh[:, c * P:(c + 1) * P],
                                         rhs=msg[:, c * F5:(c + 1) * F5],
                                         start=(c == 0), stop=(c == C - 1))
                    xo = sp5.tile([P, F5], MD, tag="xo5", name=f"xo_{t}")
                    nc.vector.tensor_add(out=xo[:], in0=pa[:], in1=sfb5[:])
                    nc.sync.dma_start(out=x6_d[t * P:(t + 1) * P, :], in_=xo[:])

            # ---- layer 6: scatter (orientation B) + dense, interleaved ----
            with tc.tile_pool(name="s6", bufs=2) as sp6, \
                 tc.tile_pool(name="s6w", bufs=1) as wp6, \
                 tc.tile_pool(name="s6ps", bufs=2, space="PSUM") as pp6, \
                 tc.tile_pool(name="s6pd", bufs=2, space="PSUM") as pd6:
                W6_sb = wp6.tile([F5, FO], F32)
                nc.sync.dma_start(out=W6_sb[:], in_=d["W6"][:, :])
                for t in range(T):
                    hre = sp6.tile([P, F5], MD, tag="hre6", name=f"hre6_{t}")
                    nc.sync.dma_start(out=hre[:], in_=x6_d[t * P:(t + 1) * P, :])
                    hsc = sp6.tile([P, F5], F32, tag="hsc6", name=f"hsc6_{t}")
                    nc.vector.tensor_scalar_mul(
                        out=hsc[:], in0=hre[:], scalar1=dinv2_sb[:, t:t + 1])
                    msg = sp6.tile([P, C * F5], MD, tag="msg6", name=f"m6_{t}")
                    for c in range(C):
                        nc.gpsimd.indirect_dma_start(
                            out=msg[:, c * F5:(c + 1) * F5],
                            out_offset=None,
                            in_=x6_d[:, :],
                            in_offset=bass.IndirectOffsetOnAxis(
                                ap=gsrc_sb[:, t * C + c:t * C + c + 1], axis=0),
                        )
                    oh = build_onehot(sp6, t, C, slot_m, norm_m, "oh6")
                    pg = pp6.tile([F5, P], F32, tag="pg", name=f"pg_{t}")
                    nc.tensor.matmul(out=pg[:], lhsT=hsc[:], rhs=ident_sb[:],
                                     is_transpose=True, start=True, stop=False,
                                     skip_group_check=True)
                    for c in range(C):
                        nc.tensor.matmul(out=pg[:], lhsT=msg[:, c * F5:(c + 1) * F5],
                                         rhs=oh[:, c * P:(c + 1) * P],
                                         start=False, stop=(c == C - 1),
                                         skip_group_check=True)
                    gst = sp6.tile([F5, P], F32, tag="gst", name=f"g_{t}")
                    nc.scalar.activation(out=gst[:], in_=pg[:], func=Ident, bias=0.0)
                    pf = pd6.tile([P, FO], F32, tag="pf", name=f"pf_{t}")
                    nc.tensor.matmul(out=pf[:], lhsT=gst[:], rhs=W6_sb[:],
                                     start=True, stop=True)
                    os_ = sp6.tile([P, FO], F32, tag="os", name=f"o_{t}")
                    nc.vector.tensor_add(out=os_[:], in0=pf[:], in1=b6rep_sb[:])
                    nc.sync.dma_start(out=out_d[t * P:(t + 1) * P, :], in_=os_[:])

    _split_multiwaits(nc)
    return nc


def _prepare(batch_vertices, img_features, edge_indices,
             W1, b1, W2, b2, W3, b3, W4, b4, W5, b5, W6, b6):
    B, N, _ = batch_vertices.shape
    FM = W1.shape[1]
    F5 = W5.shape[1]
    FO = W6.shape[1]

    ei = np.asarray(edge_indices).astype(np.int64)
    g = _pack_graph(ei[0], ei[1], N)
    NP, T, C, perm = g["NP"], g["T"], g["C"], g["perm"]

    KM = FM // P
    hc = img_features.astype(np.float32) @ W1[3:].astype(np.float32)

    valid = perm >= 0
    vperm = np.zeros((B, NP, 3), np.float32)
    vperm[:, valid, :] = batch_vertices[:, perm[valid], :]

    common = {
        "W1v": np.ascontiguousarray(W1[:3].astype(np.float32)),
        "W2": np.ascontiguousarray(W2.astype(np.float32)),
        "W3": np.ascontiguousarray(W3.astype(np.float32)),
        "W4": np.ascontiguousarray(W4.astype(np.float32)),
        "W5": np.ascontiguousarray(W5.astype(np.float32)),
        "W6": np.ascontiguousarray(W6.astype(np.float32)),
        "B14": np.ascontiguousarray(
            np.stack([b.reshape(KM, P).T for b in (b1, b2, b3, b4)],
                     axis=1).reshape(P, 4 * KM).astype(np.float32)),
        "b5rep": np.tile(b5.astype(np.float32), (P, 1)),
        "b6rep": np.tile(b6.astype(np.float32), (P, 1)),
        "gsrc": g["gsrc"], "slot": g["slot"], "normv": g["norm"],
        "B14R": np.ascontiguousarray(
            np.tile(np.concatenate([b1, b2, b3, b4]).astype(np.float32), (P, 1))),
        "dinv2": g["dinv2"],
    }
    in_maps = []
    for b in range(B):
        m = dict(common)
        m["xT1"] = np.ascontiguousarray(vperm[b].T)
        m["hcrep"] = np.tile(hc[b], (P, 1))
        in_maps.append(m)
    meta = dict(NP=NP, T=T, C=C, perm=perm, valid=valid, B=B, N=N,
                FM=FM, F5=F5, FO=FO)
    return in_maps, meta


_BUILD_CACHE = {}


def run(inputs, trace=False):
    in_maps, meta = _prepare(**inputs)
    key = (meta["NP"], meta["C"], meta["FM"], meta["F5"], meta["FO"])
    if key not in _BUILD_CACHE:
        t0 = time.time()
        _BUILD_CACHE[key] = _build_nc(meta["NP"], meta["T"], meta["C"],
                                      meta["FM"], meta["F5"], meta["FO"])
        print(f"[kernel] built bass program in {time.time()-t0:.1f}s", file=sys.stderr)
    nc = _BUILD_CACHE[key]
    B = meta["B"]
    res = run_bass_kernel_spmd(nc, in_maps, core_ids=list(range(B)), trace=trace)
    perm, valid, N = meta["perm"], meta["valid"], meta["N"]
    out = np.empty((B, N, meta["FO"]), np.float32)
    for b in range(B):
        dev = res.results[b]["out"]
        out[b, perm[valid], :] = dev[valid, :]
    return out, res


def kernel(**inputs) -> np.ndarray:
    out, _ = run(inputs)
    return out
